# revision 1
# baseline (speedup 1.0000x reference)
"""Trainium2 Bass kernel for AttentionM (dense transformer block).

Computes, for x [4, 2048, 1024] and q/k/v CSS-gated projections:
    q = (x@Wq+bq)*sigmoid(x@Wqc+bqc)   -> [B, Sp, 16 heads, 16]
    k = likewise                        -> [B, Sp, 16, 16]
    v = likewise (64-wide heads)        -> [B, Sp, 16, 64]
    ctx = softmax(q k^T / 8) v          -> [B, S, 1024]
with Sp = S+16 zero-padded rows (pad tokens participate via bias-only css).

Sharding over 8 NeuronCores: 4-way data parallel over batch x 2-way tensor
parallel over heads (8 heads per core). Each core gets x[b] (padded) and its
head-slice of the weights, computes ctx[b, :, hg*512:(hg+1)*512].

Per-core dataflow (all matmuls in float32r, 1 cycle/row at N>=256):
  1. x streams in four 512-column blocks; each block is PE-transposed to a
     column-range xT part, immediately feeding that block's kT/qT projection
     chunks (feature-major [128 = 8h x 16, seq], sigmoid gate fused via one
     ACT op + one DVE scalar_tensor_tensor) while later blocks are still in
     flight from HBM. v tiles (token-major [tok, 8, 64+1], bias added via a
     K=1 ones-column matmul) lag one block behind so the PE never waits.
  2. The 16 identical zero-pad k rows collapse into one rank-1 update:
     block-diag pad-k [128, 8] batches all heads' pad scores into two
     [8, 1024] matmuls and exps (ln 16 folded into the ACT bias), restaged to
     partition 0. v'_pad = bv*sigmoid(bvc) directly (pad x rows are zero).
  3. Attention per (qr in 2, h in 8): scoresT [k_tile, q] = kh^T qh on PE
     (kh/qh restaged to partition 0 by SBUF->SBUF DMA -- matmul operands must
     share a 32-aligned base partition); exp on ACT straight out of PSUM
     (scale=1/8, no max subtraction -- scores are provably in [-3, 3]);
     ctxT [65, q] accumulated over k tiles in PSUM, with row 64 = exp @ ones
     as the softmax denominator. The PSUM group is opened by the dep-free pad
     rank-1 update and the ctx matmuls run three k-tiles behind the scores, so
     the ACT exp stream (the bottleneck: 1038ns per [128,1024] exp) runs
     back-to-back.
  4. Epilogue per head (deferred into the next head's k-loop): PE-transpose
     ctxT to token-major, DVE reciprocal of the denominator column, multiply,
     and merged output DMAs (heads 0..6 flush during the last head's loop).
"""

import sys

if "/opt/trn_rl_repo" not in sys.path:
    sys.path.insert(0, "/opt/trn_rl_repo")

import numpy as np

import concourse.bacc as bacc
import concourse.mybir as mybir
import concourse.tile as tile
from concourse.bass_utils import run_bass_kernel_spmd
from concourse.masks import make_identity

F32 = mybir.dt.float32
F32R = mybir.dt.float32r
AF = mybir.ActivationFunctionType
ALU = mybir.AluOpType

B = 4
S = 2048          # real sequence
PAD = 16
SP = S + PAD      # padded sequence (k extent)
D = 1024
DC = D // 128     # 8 contraction chunks
HL = 8            # heads per core
QL = 16           # q/k head dim
VL = 64           # v head dim
NKT = SP // 128 + 1          # 17 k tiles (16 full + 16-row tail)
QR = 1024                    # q range per psum accumulator
SCALE = 1.0 / 8.0            # 1/sqrt(64)


def _build(repeat=1):
    nc = bacc.Bacc("TRN2", target_bir_lowering=False, debug=False, num_devices=8)

    x_d = nc.dram_tensor("x", [SP, D], F32R, kind="ExternalInput").ap()
    idr_d = nc.dram_tensor("idr", [128, 128], F32R, kind="ExternalInput").ap()
    bdiag_d = nc.dram_tensor("bdiag", [128, 8], F32R, kind="ExternalInput").ap()
    wq_d = nc.dram_tensor("wq", [D, 128], F32R, kind="ExternalInput").ap()
    wqc_d = nc.dram_tensor("wqc", [D, 128], F32R, kind="ExternalInput").ap()
    wk_d = nc.dram_tensor("wk", [D, 128], F32R, kind="ExternalInput").ap()
    wkc_d = nc.dram_tensor("wkc", [D, 128], F32R, kind="ExternalInput").ap()
    wv_d = nc.dram_tensor("wv", [D, 512], F32R, kind="ExternalInput").ap()
    wvc_d = nc.dram_tensor("wvc", [D, 512], F32R, kind="ExternalInput").ap()
    bq_d = nc.dram_tensor("bq", [128], F32, kind="ExternalInput").ap()
    bqc_d = nc.dram_tensor("bqc", [128], F32, kind="ExternalInput").ap()
    bk_d = nc.dram_tensor("bk", [128], F32, kind="ExternalInput").ap()
    bkc_d = nc.dram_tensor("bkc", [128], F32, kind="ExternalInput").ap()
    bv_d = nc.dram_tensor("bv", [512], F32R, kind="ExternalInput").ap()
    bvc_d = nc.dram_tensor("bvc", [512], F32R, kind="ExternalInput").ap()
    y_d = nc.dram_tensor("y", [S, 512], F32, kind="ExternalOutput").ap()

    with tile.TileContext(nc) as tc:
        for _ in range(repeat):
            _emit(nc, tc, x_d, idr_d, bdiag_d, wq_d, wqc_d, wk_d, wkc_d, wv_d,
                  wvc_d, bq_d, bqc_d, bk_d, bkc_d, bv_d, bvc_d, y_d)
    nc.compile()
    return nc


def _emit(nc, tc, x_d, idr_d, bdiag_d, wq_d, wqc_d, wk_d, wkc_d, wv_d,
          wvc_d, bq_d, bqc_d, bk_d, bkc_d, bv_d, bvc_d, y_d):
    # ---------------- long-lived pools ----------------
    const = tc.alloc_tile_pool(name="const", bufs=1)
    proj = tc.alloc_tile_pool(name="proj", bufs=1)
    padp = tc.alloc_tile_pool(name="padp", bufs=1)
    pp_mm = tc.alloc_tile_pool(name="pp_mm", bufs=2, space="PSUM")
    pp_tp = tc.alloc_tile_pool(name="pp_tp", bufs=2, space="PSUM")
    pp_acc = tc.alloc_tile_pool(name="pp_acc", bufs=1, space="PSUM")

    ph13 = tc.alloc_tile_pool(name="ph13", bufs=1)
    # xT split by column range so consumers start before all of x is transposed
    xT_parts = [ph13.tile([128, DC, 528 if i == 3 else 512], F32R, name=f"xT{i}")
                for i in range(4)]

    def xTs(d, c0, csz):
        part = min(c0 // 512, 3)
        lo = c0 - part * 512
        assert lo + csz <= (528 if part == 3 else 512)
        return xT_parts[part][:, d, lo:lo + csz]
    wpool = tc.alloc_tile_pool(name="wpool", bufs=1)
    ph1 = tc.alloc_tile_pool(name="ph1", bufs=4)

    # f32r identity first: the x transposes block on it
    idr = const.tile([128, 128], F32R, name="idr")
    nc.sync.dma_start(out=idr, in_=idr_d)
    ident = const.tile([128, 128], F32)
    make_identity(nc, ident)

    # ---- phases 1+2 interleaved: x-block transposes feed q/k chunks ----
    # Block b = x tiles 4b..4b+3 (block 3 also takes the 16-row pad tile).
    # After block b's transposes, the kT and qT projection chunks for column
    # range [512b, 512(b+1)) are emitted, so the PE works on projections while
    # the next x block is still streaming from HBM.
    def ppart(bias_d, dtype=F32):
        t = const.tile([128, 1], dtype, name=f"b_{bias_d.name}")
        nc.sync.dma_start(out=t, in_=bias_d.unsqueeze(-1))
        return t

    wq = wpool.tile([128, DC, 128], F32R, name="wq")
    wqc = wpool.tile([128, DC, 128], F32R, name="wqc")
    wk = wpool.tile([128, DC, 128], F32R, name="wk")
    wkc = wpool.tile([128, DC, 128], F32R, name="wkc")
    wv = wpool.tile([128, DC, 512], F32R, name="wv")
    wvc = wpool.tile([128, DC, 512], F32R, name="wvc")

    qT = proj.tile([128, S], F32R, name="qT")       # [8h*16, q]
    kT = proj.tile([128, SP], F32R, name="kT")      # [8h*16, k]
    vt = proj.tile([128, NKT, HL, VL + 1], F32R, name="vt")  # token-major v + ones

    ones_col = const.tile([1, 128], F32R, name="ones_col")

    # pre-attention phases rotate a third psum slot through the (idle)
    # attention accumulator pool for deeper pipelining
    _rr = [0]

    def mm_tile():
        _rr[0] += 1
        if _rr[0] % 3 == 0:
            return pp_acc.tile([128, QR], F32, name="acc")
        return pp_mm.tile([128, 1024], F32, name="mm")

    sig2 = tc.alloc_tile_pool(name="sig2", bufs=2)
    sig3 = tc.alloc_tile_pool(name="sig3", bufs=2)
    emit_v_tile_fn = [None]

    def emit_x_tile(t):
        tsz = min(128, SP - t * 128)
        xt = ph1.tile([128, D], F32R, name="xload")
        nc.sync.dma_start(out=xt[0:tsz, :], in_=x_d[t * 128: t * 128 + tsz, :])
        return xt, tsz

    def emit_tposes(t, xt, tsz):
        for half in range(2):
            tp = pp_tp.tile([128, 512], F32R, name="tp")
            for jj in range(4):
                d = half * 4 + jj
                nc.tensor.transpose(
                    out=tp[:, jj * 128: jj * 128 + tsz],
                    in_=xt[0:tsz, d * 128:(d + 1) * 128],
                    identity=idr[0:tsz, 0:tsz],
                )
            part = min(t // 4, 3)
            lo = t * 128 - part * 512
            nc.vector.tensor_copy(
                out=xT_parts[part][:, half * 4:(half + 1) * 4, lo:lo + tsz],
                in_=tp.rearrange("p (b c) -> p b c", b=4)[:, :, 0:tsz],
            )

    def emit_v_tile(t):
        """v'[t] = [(lin+bv) * sigmoid(linc+bvc) | 1], token-major."""
        tsz = min(128, SP - t * 128)
        tc0 = t * 128
        ps = mm_tile()
        for d in range(DC):
            nc.tensor.matmul(ps[0:tsz, 0:512], xTs(d, tc0, tsz), wv[:, d, :],
                             start=(d == 0), stop=False)
        nc.tensor.matmul(ps[0:tsz, 0:512], ones_col[:, 0:tsz], bv_row,
                         start=False, stop=True)
        for d in range(DC):
            nc.tensor.matmul(ps[0:tsz, 512:1024], xTs(d, tc0, tsz),
                             wvc[:, d, :], start=(d == 0), stop=False)
        nc.tensor.matmul(ps[0:tsz, 512:1024], ones_col[:, 0:tsz], bvc_row,
                         start=False, stop=True)
        sg = sig3.tile([128, 512], F32, name="sigv")
        nc.scalar.activation(out=sg[0:tsz, :], in_=ps[0:tsz, 512:1024],
                             func=AF.Sigmoid)
        nc.vector.tensor_tensor(
            out=vt[0:tsz, t, :, 0:VL],
            in0=ps[0:tsz, 0:512].rearrange("p (h v) -> p h v", h=HL),
            in1=sg[0:tsz, :].rearrange("p (h v) -> p h v", h=HL),
            op=ALU.mult)
        nc.scalar.activation(out=vt[:, t, :, VL:VL + 1],
                             in_=idr[:, 0:HL].unsqueeze(-1),
                             func=AF.Copy, scale=0.0, bias=1.0)
    emit_v_tile_fn[0] = emit_v_tile

    def emit_qk_chunk(wl, wcl, bl, bcl, dest, c0, csz):
        ps = mm_tile()
        for d in range(DC):
            nc.tensor.matmul(ps[:, 0:csz], wl[:, d, :], xTs(d, c0, csz),
                             start=(d == 0), stop=(d == DC - 1))
        for d in range(DC):
            nc.tensor.matmul(ps[:, 512:512 + csz], wcl[:, d, :],
                             xTs(d, c0, csz),
                             start=(d == 0), stop=(d == DC - 1))
        sig = sig2.tile([128, 512], F32, name="sig")
        nc.scalar.activation(out=sig[:, 0:csz], in_=ps[:, 512:512 + csz],
                             func=AF.Sigmoid, bias=bcl)
        nc.vector.scalar_tensor_tensor(
            out=dest[:, c0:c0 + csz], in0=ps[:, 0:csz], scalar=bl,
            in1=sig[:, 0:csz], op0=ALU.add, op1=ALU.mult)

    bias_sbs = None
    emitted_v = 0
    for blk in range(4):
        ts_in_blk = range(12, NKT) if blk == 3 else range(blk * 4, blk * 4 + 4)
        for t in ts_in_blk:
            xt, tsz = emit_x_tile(t)
            if bias_sbs is None:
                bq_sb = ppart(bq_d)
                bqc_sb = ppart(bqc_d)
                bk_sb = ppart(bk_d)
                bkc_sb = ppart(bkc_d)
                bv_row = const.tile([1, 512], F32R, name="bv_row")
                nc.sync.dma_start(out=bv_row, in_=bv_d.unsqueeze(0))
                bvc_row = const.tile([1, 512], F32R, name="bvc_row")
                nc.sync.dma_start(out=bvc_row, in_=bvc_d.unsqueeze(0))
                nc.scalar.activation(out=ones_col, in_=idr[0:1, :], func=AF.Copy,
                                     scale=0.0, bias=1.0)
                bias_sbs = True
            emit_tposes(t, xt, tsz)
        if blk == 0:
            # qk weights after the first x block: needed from ~10us on, and
            # they must not delay the x stream that gates the transposes
            for w_sb, w_dd in ((wk, wk_d), (wkc, wkc_d), (wq, wq_d), (wqc, wqc_d)):
                nc.sync.dma_start(out=w_sb,
                                  in_=w_dd.rearrange("(a p) c -> p a c", p=128))
        if blk == 1:
            # v weights after the second x block; v tiles lag one block so
            # these arrive before the first v matmuls need them
            wrv = wv_d.rearrange("(a p) c -> p a c", p=128)
            wrvc = wvc_d.rearrange("(a p) c -> p a c", p=128)
            for d in range(DC):
                nc.sync.dma_start(out=wv[:, d, :], in_=wrv[:, d, :])
                nc.sync.dma_start(out=wvc[:, d, :], in_=wrvc[:, d, :])
        emit_qk_chunk(wk, wkc, bk_sb, bkc_sb, kT, blk * 512, 512)
        emit_qk_chunk(wq, wqc, bq_sb, bqc_sb, qT, blk * 512, 512)
        if blk >= 1:
            while emitted_v < blk * 4:
                emit_v_tile_fn[0](emitted_v)
                emitted_v += 1
    emit_qk_chunk(wk, wkc, bk_sb, bkc_sb, kT, S, PAD)
    # The 16 identical zero-pad k rows collapse into one rank-1 update:
    # acc += exp(s_pad/8 + ln 16) * v'_pad. Build block-diag pad-k [128, 8]
    # (head h's pad-k vector at rows 16h..16h+16), batch all heads' pad
    # scores into two [8, 1024] matmuls + one exp each, then restage to
    # partition 0 for the K=1 ctx update.
    padk = padp.tile([128, HL], F32R, name="padk")
    bdiag = padp.tile([128, HL], F32R, name="bdiag")
    nc.sync.dma_start(out=bdiag, in_=bdiag_d)
    nc.vector.tensor_scalar(out=padk, in0=bdiag,
                            scalar1=kT[:, S:S + 1].bitcast(F32),
                            scalar2=None, op0=ALU.mult)
    e_pad = padp.tile([HL, 2, QR], F32R, name="e_pad")
    LN16 = float(np.log(16.0))
    ln16_sb = padp.tile([128, 1], F32, name="ln16")
    nc.scalar.activation(out=ln16_sb, in_=idr[:, 0:1], func=AF.Copy,
                         scale=0.0, bias=LN16)
    for r in range(2):
        pps = pp_mm.tile([128, QR], F32, name="mm")
        for j in range(QR // 512):
            nc.tensor.matmul(pps[0:HL, j * 512:(j + 1) * 512], padk,
                             qT[:, r * QR + j * 512: r * QR + (j + 1) * 512],
                             start=True, stop=True)
        nc.scalar.activation(out=e_pad[:, r, :], in_=pps[0:HL, :],
                             func=AF.Exp, scale=SCALE, bias=ln16_sb[0:HL, :])

    while emitted_v < NKT - 1:
        emit_v_tile_fn[0](emitted_v)
        emitted_v += 1

    # (v tiles are emitted inside the block loop above via emit_v_tile_fn)

    # pad rows of x are zero, so v'_pad = bv * sigmoid(bvc) -- no matmul needed.
    # All 16 pad rows are identical; only row 0 is kept (used as a K=1 lhsT).
    sgp = sig3.tile([128, 512], F32, name="sigv")
    nc.scalar.activation(out=sgp[0:1, :], in_=bvc_row.bitcast(F32), func=AF.Sigmoid)
    nc.vector.tensor_tensor(
        out=vt[0:1, NKT - 1, :, 0:VL],
        in0=bv_row.bitcast(F32).rearrange("p (h v) -> p h v", h=HL),
        in1=sgp[0:1, :].rearrange("p (h v) -> p h v", h=HL),
        op=ALU.mult)
    nc.scalar.activation(out=vt[0:1, NKT - 1, :, VL:VL + 1],
                         in_=idr[0:1, 0:HL].unsqueeze(-1),
                         func=AF.Copy, scale=0.0, bias=1.0)
    sig3.release()
    sig2.release()
    ph1.release()
    wpool.release()
    ph13.release()

    # ---------------- phase 4: attention ----------------
    stage = tc.alloc_tile_pool(name="stage", bufs=2)
    expp = tc.alloc_tile_pool(name="expp", bufs=5)
    ctp = tc.alloc_tile_pool(name="ctp", bufs=2)
    outp = tc.alloc_tile_pool(name="outp", bufs=2)
    rcp = tc.alloc_tile_pool(name="rcp", bufs=2)

    def stage_head(qr, h):
        q0 = qr * QR
        qh = stage.tile([QL, QR], F32R, name="qh")
        nc.sync.dma_start(out=qh, in_=qT[h * QL:(h + 1) * QL, q0:q0 + QR])
        kh = stage.tile([QL, S], F32R, name="kh")
        nc.sync.dma_start(out=kh, in_=kT[h * QL:(h + 1) * QL, 0:S])
        ep = stage.tile([1, QR], F32R, name="ep")
        nc.sync.dma_start(out=ep, in_=e_pad[h:h + 1, qr, :])
        return qh, kh, ep

    def head_loop(qr, h, qh, kh, ep, epi=None):
        """scores/exp/ctx over 16 full k tiles; ctx pipelined two k-tiles
        behind the scores so the exp chain never waits on semaphores; the
        pad block lands as a final K=1 rank-1 update. epi (the previous
        head's epilogue, as a generator) is consumed one q-tile per k-tile
        so its PE transposes hide inside the ACT-bound slack."""
        acc = pp_acc.tile([128, QR], F32, name="acc")
        # the pad-block rank-1 update depends on no exp: open the psum
        # accumulation group with it at head start (start=True), freeing the
        # head's tail of everything but the last two ctx flushes
        for j in range(QR // 512):
            nc.tensor.matmul(
                acc[0:VL + 1, j * 512:(j + 1) * 512],
                vt[0:1, NKT - 1, h, :],
                ep[0:1, j * 512:(j + 1) * 512],
                start=True, stop=False)
        pend = []  # (et, t) whose ctx matmuls are not yet emitted
        for t in range(NKT - 1):
            if t >= 5 and epi is not None:
                next(epi, None)
            sc = pp_mm.tile([128, QR], F32, name="mm")
            for j in range(QR // 512):
                nc.tensor.matmul(
                    sc[:, j * 512:(j + 1) * 512],
                    kh[:, t * 128:(t + 1) * 128],
                    qh[:, j * 512:(j + 1) * 512],
                    start=True, stop=True)
            et = expp.tile([128, QR], F32R, name="et")
            nc.scalar.activation(out=et, in_=sc, func=AF.Exp, scale=SCALE)
            pend.append((et, t))
            if len(pend) > 3:
                _emit_ctx(acc, h, *pend.pop(0))
        for p in pend:
            _emit_ctx(acc, h, *p)
        return acc

    def _emit_ctx(acc, h, et, t):
        for j in range(QR // 512):
            nc.tensor.matmul(
                acc[0:VL + 1, j * 512:(j + 1) * 512],
                vt[0:128, t, h, :],
                et[:, j * 512:(j + 1) * 512],
                start=False, stop=(t == NKT - 2))

    def epilogue_copy(acc):
        ct = ctp.tile([VL + 1, QR], F32, name="ct")
        nc.vector.tensor_copy(out=ct, in_=acc[0:VL + 1, :])
        return ct

    def head_epilogue(qr, h, ct, out_sb, dma=False):
        last = h == HL - 1
        for qt in range(QR // 128):
            qsl = slice(qt * 128, (qt + 1) * 128)
            tp = pp_tp.tile([128, 512], F32R, name="tp")
            tpf = tp.bitcast(F32)
            nc.tensor.transpose(
                out=tpf[:, 0:VL + 1],
                in_=ct[:, qsl],
                identity=ident[0:VL + 1, 0:VL + 1])
            rc = rcp.tile([128, 1], F32, name="rc")
            nc.vector.reciprocal(out=rc, in_=tpf[:, VL:VL + 1])
            nc.vector.tensor_scalar_mul(
                out_sb[:, qt, h * VL:(h + 1) * VL], tpf[:, 0:VL], rc)
            yield
        r0 = qr * QR
        yr = y_d[r0:r0 + QR, :].rearrange("(a p) c -> p a c", p=128)
        if h == HL - 2:
            # heads 0..6 are final in cols [0:448): flush them in one DMA
            # during the last head's k-loop so the tail only moves 64 columns
            nc.sync.dma_start(out=yr[:, :, 0:(HL - 1) * VL],
                              in_=out_sb[:, :, 0:(HL - 1) * VL])
        elif last:
            nc.sync.dma_start(out=yr[:, :, (HL - 1) * VL:],
                              in_=out_sb[:, :, (HL - 1) * VL:])
        yield

    # flat (qr, h) stream: every head's epilogue (transposes + divide + DMA)
    # is deferred into the NEXT head's k-loop, including across the qr
    # boundary; only the global last runs inline
    out_sbs = [outp.tile([128, QR // 128, 512], F32, name="out_sb")
               for _ in range(S // QR)]
    prev = None                              # (qr, h, ct) awaiting epilogue
    for qr in range(S // QR):               # 2 q ranges of 1024
        for h in range(HL):
            qh, kh, ep = stage_head(qr, h)
            epi = None
            if prev is not None:
                epi = head_epilogue(prev[0], prev[1], prev[2], out_sbs[prev[0]])
            acc = head_loop(qr, h, qh, kh, ep, epi=epi)
            if epi is not None:
                for _ in epi:
                    pass
            # emit the wide psum->sbuf copy for THIS head immediately (its acc
            # is complete), so the next head's epilogue transposes never wait
            ct = epilogue_copy(acc)
            prev = (qr, h, ct)
    for _ in head_epilogue(prev[0], prev[1], prev[2], out_sbs[prev[0]], dma=True):
        pass

    for p in (rcp, outp, ctp, expp, stage, pp_acc, pp_tp, pp_mm,
              padp, proj, const):
        p.release()


_NC = None


def _get_nc():
    global _NC
    if _NC is None:
        _NC = _build()
    return _NC


def _shard_inputs(inputs):
    x = np.ascontiguousarray(np.asarray(inputs["x"], dtype=np.float32))
    pad = np.zeros((PAD, D), np.float32)
    ident = np.eye(128, dtype=np.float32)
    bdiag = np.repeat(np.eye(8, dtype=np.float32), 16, axis=0)
    in_maps = []
    for c in range(8):
        b, hg = c // 2, c % 2
        qk = slice(hg * 128, (hg + 1) * 128)
        vv = slice(hg * 512, (hg + 1) * 512)
        in_maps.append({
            "x": np.ascontiguousarray(np.concatenate([x[b], pad], axis=0)),
            "idr": ident,
            "bdiag": bdiag,
            "wq": np.ascontiguousarray(inputs["Wq"][:, qk]),
            "wqc": np.ascontiguousarray(inputs["Wqc"][:, qk]),
            "wk": np.ascontiguousarray(inputs["Wk"][:, qk]),
            "wkc": np.ascontiguousarray(inputs["Wkc"][:, qk]),
            "wv": np.ascontiguousarray(inputs["Wv"][:, vv]),
            "wvc": np.ascontiguousarray(inputs["Wvc"][:, vv]),
            "bq": np.ascontiguousarray(inputs["bq"][qk]),
            "bqc": np.ascontiguousarray(inputs["bqc"][qk]),
            "bk": np.ascontiguousarray(inputs["bk"][qk]),
            "bkc": np.ascontiguousarray(inputs["bkc"][qk]),
            "bv": np.ascontiguousarray(inputs["bv"][vv]),
            "bvc": np.ascontiguousarray(inputs["bvc"][vv]),
        })
    return in_maps


def kernel(**inputs) -> np.ndarray:
    nc = _get_nc()
    in_maps = _shard_inputs(inputs)
    res = run_bass_kernel_spmd(nc, in_maps, list(range(8)))
    out = np.empty((B, S, 1024), np.float32)
    for c in range(8):
        b, hg = c // 2, c % 2
        out[b, :, hg * 512:(hg + 1) * 512] = res.results[c]["y"]
    return out


if __name__ == "__main__":
    rng = np.random.default_rng(0)
    d = 1.0 / np.sqrt(D)
    inputs = {
        "x": rng.standard_normal((B, S, D), dtype=np.float32),
        "Wq": rng.standard_normal((D, 256), dtype=np.float32) * d,
        "bq": rng.standard_normal(256).astype(np.float32) * 0.02,
        "Wqc": rng.standard_normal((D, 256), dtype=np.float32) * d,
        "bqc": rng.standard_normal(256).astype(np.float32) * 0.02,
        "Wk": rng.standard_normal((D, 256), dtype=np.float32) * d,
        "bk": rng.standard_normal(256).astype(np.float32) * 0.02,
        "Wkc": rng.standard_normal((D, 256), dtype=np.float32) * d,
        "bkc": rng.standard_normal(256).astype(np.float32) * 0.02,
        "Wv": rng.standard_normal((D, 1024), dtype=np.float32) * d,
        "bv": rng.standard_normal(1024).astype(np.float32) * 0.02,
        "Wvc": rng.standard_normal((D, 1024), dtype=np.float32) * d,
        "bvc": rng.standard_normal(1024).astype(np.float32) * 0.02,
    }
    y = kernel(**inputs)
    print("kernel output", y.shape, y.dtype, float(np.abs(y).max()))



# revision 26
# speedup vs baseline: 1.0187x; 1.0187x over previous
"""Trainium2 Bass kernel for AttentionM (dense transformer block).

Computes, for x [4, 2048, 1024] and q/k/v CSS-gated projections:
    q = (x@Wq+bq)*sigmoid(x@Wqc+bqc)   -> [B, Sp, 16 heads, 16]
    k = likewise                        -> [B, Sp, 16, 16]
    v = likewise (64-wide heads)        -> [B, Sp, 16, 64]
    ctx = softmax(q k^T / 8) v          -> [B, S, 1024]
with Sp = S+16 zero-padded rows. Pad x rows are zero, so pad k/v collapse to
bias-only constants; the 16 identical pad rows enter as one rank-1 update with
ln16 folded into the exp bias.

Sharding over 8 NeuronCores: 4-way data parallel over batch x 2-way tensor
parallel over heads (8 heads per core). Each core gets x[b] (pre-transposed on
the host to feature-major, so no on-chip transposes are needed and all loads
are contiguous) plus its head-slice of the weights, and computes
ctx[b, :, hg*512:(hg+1)*512].

Per-core schedule (the ACT exp stream is the roofline: 256 exps of [128,1024]
at ~1038ns = 266us; everything else hides under it):
  - Prologue (~30us): xT streams block-by-block through a 1-slot rolling
    window feeding the k and q projection chunks (sigmoid gates, all emitted
    before the first exp so the ACT table switches exactly once). Pad-k/pad-v
    come from biases alone. e_pad for both passes; heads whose 16h base
    partition is not in {0,32,64} are restaged (matmul tile_position rule).
  - Two q-passes of 1024 cols; k-tiles loop OUTER, heads inner: scores
    sc[128k,1024q] (f32r, K=16) -> exp on ACT -> bf16 et; ctx partial
    pt[128q, 65] = et^T @ vt per 128-q chunk (bf16, N=65 -> 27ns/matmul;
    vt column 64 is ones so the softmax denominator rides along) -> Pool
    drains psum into the f32 ctx_sb accumulator. Ctx lags the exp stream.
  - v projections pipeline INSIDE pass-0's k-loop one slot ahead of use,
    re-streaming xT tiles through a 2-slot window; their sigmoid gates are
    computed as 1/(1+exp(-z)) (ACT exp + Pool add + DVE reciprocal + Pool
    mult) so ACT never reloads the sigmoid table mid-stream.
  - Epilogue per head: DVE reciprocal of the denominator column, Pool
    multiply straight out of ctx_sb into bf16 out_sb, merged output DMAs.
"""

import sys

if "/opt/trn_rl_repo" not in sys.path:
    sys.path.insert(0, "/opt/trn_rl_repo")

import numpy as np

import concourse.bacc as bacc
import concourse.mybir as mybir
import concourse.tile as tile
from concourse.bass_utils import run_bass_kernel_spmd

F32 = mybir.dt.float32
F32R = mybir.dt.float32r
BF16 = mybir.dt.bfloat16
AF = mybir.ActivationFunctionType
ALU = mybir.AluOpType

B = 4
S = 2048          # real sequence per core
D = 1024
DC = D // 128     # 8 contraction chunks
HL = 8            # heads per core
QL = 16           # q/k head dim
VL = 64           # v head dim
NT = S // 128     # 16 k tiles
NP = 2            # q passes
QW = S // NP      # 1024 q cols per pass
NC_ = QW // 128   # 8 q chunks per pass
SCALE = 1.0 / 8.0
LN16 = float(np.log(16.0))
CTX_LAG = 2


def _build(repeat=1):
    nc = bacc.Bacc("TRN2", target_bir_lowering=False, debug=False, num_devices=8)

    xT_d = nc.dram_tensor("xT", [D, S], F32R, kind="ExternalInput").ap()
    bdiag_d = nc.dram_tensor("bdiag", [128, 8], F32R, kind="ExternalInput").ap()
    wq_d = nc.dram_tensor("wq", [D, 128], F32R, kind="ExternalInput").ap()
    wqc_d = nc.dram_tensor("wqc", [D, 128], F32R, kind="ExternalInput").ap()
    wk_d = nc.dram_tensor("wk", [D, 128], F32R, kind="ExternalInput").ap()
    wkc_d = nc.dram_tensor("wkc", [D, 128], F32R, kind="ExternalInput").ap()
    wv_d = nc.dram_tensor("wv", [D, 512], F32R, kind="ExternalInput").ap()
    wvc_d = nc.dram_tensor("wvc", [D, 512], F32R, kind="ExternalInput").ap()
    bq_d = nc.dram_tensor("bq", [128], F32, kind="ExternalInput").ap()
    bqc_d = nc.dram_tensor("bqc", [128], F32, kind="ExternalInput").ap()
    bk_d = nc.dram_tensor("bk", [128], F32, kind="ExternalInput").ap()
    bkc_d = nc.dram_tensor("bkc", [128], F32, kind="ExternalInput").ap()
    bv_d = nc.dram_tensor("bv", [512], F32R, kind="ExternalInput").ap()
    bvc_d = nc.dram_tensor("bvc", [512], F32R, kind="ExternalInput").ap()
    y_d = nc.dram_tensor("y", [S, 512], BF16, kind="ExternalOutput").ap()

    with tile.TileContext(nc) as tc:
        for _ in range(repeat):
            _emit(nc, tc, xT_d, bdiag_d, wq_d, wqc_d, wk_d, wkc_d, wv_d,
                  wvc_d, bq_d, bqc_d, bk_d, bkc_d, bv_d, bvc_d, y_d)
    nc.compile()
    return nc


def _emit(nc, tc, xT_d, bdiag_d, wq_d, wqc_d, wk_d, wkc_d, wv_d,
          wvc_d, bq_d, bqc_d, bk_d, bkc_d, bv_d, bvc_d, y_d):
    # xT_d viewed as [128 dpart, 8 dchunk, S]
    xTr = xT_d.rearrange("(c d) t -> d c t", d=128)

    # ---------------- pools (SBUF release order is LIFO) ----------------
    const = tc.alloc_tile_pool(name="const", bufs=1)
    proj = tc.alloc_tile_pool(name="proj", bufs=1)
    vtp = tc.alloc_tile_pool(name="vtp", bufs=1)
    epadp = tc.alloc_tile_pool(name="epadp", bufs=1)
    sigp = tc.alloc_tile_pool(name="sigp", bufs=3)
    stg = tc.alloc_tile_pool(name="stg", bufs=1)
    expp = tc.alloc_tile_pool(name="expp", bufs=3)
    ctxp = tc.alloc_tile_pool(name="ctxp", bufs=1)
    rcp = tc.alloc_tile_pool(name="rcp", bufs=2)
    outp = tc.alloc_tile_pool(name="outp", bufs=1)
    # released at end of pass 0
    wvpool = tc.alloc_tile_pool(name="wvpool", bufs=1)
    xTroll = tc.alloc_tile_pool(name="xTroll", bufs=1)
    # etmp released shortly after the prologue, wpool at its end
    etmp = tc.alloc_tile_pool(name="etmp", bufs=1)
    wpool = tc.alloc_tile_pool(name="wpool", bufs=1)

    pp_mm = tc.alloc_tile_pool(name="pp_mm", bufs=2, space="PSUM")
    pp_ctxa = tc.alloc_tile_pool(name="pp_ctxa", bufs=1, space="PSUM")
    pp_ctxb = tc.alloc_tile_pool(name="pp_ctxb", bufs=1, space="PSUM")
    pp_v = tc.alloc_tile_pool(name="pp_v", bufs=1, space="PSUM")

    wq = wpool.tile([128, DC, 128], F32R, name="wq")
    wqc = wpool.tile([128, DC, 128], F32R, name="wqc")
    wk = wpool.tile([128, DC, 128], F32R, name="wk")
    wkc = wpool.tile([128, DC, 128], F32R, name="wkc")
    wv = wvpool.tile([128, DC, 512], F32R, name="wv")
    wvc = wvpool.tile([128, DC, 512], F32R, name="wvc")

    qT = proj.tile([128, S], F32R, name="qT")       # [8h*16, q]
    kT = proj.tile([128, S], F32R, name="kT")       # [8h*16, k]
    # token-major v + ones column; index NT (=16) holds the pad row
    vt = vtp.tile([128, NT + 1, HL, VL + 1], BF16, name="vt")
    e_pad = etmp.tile([128, NP, QW], BF16, name="e_pad")   # [8h, p, q]
    # pad-exp staged for K=1 lhsT use (base partitions limited to 0/32/64):
    # head h at partition 32*(h%3), free index h//3; slot rotated per pass
    ept = {}

    ones_col = const.tile([1, 128], F32R, name="ones_col")
    ln16_sb = const.tile([128, 1], F32, name="ln16")
    padk = const.tile([128, HL], F32R, name="padk")
    bdiag = const.tile([128, HL], F32R, name="bdiag")
    nc.gpsimd.memset(ones_col.bitcast(F32), 1.0)
    nc.gpsimd.memset(ln16_sb, LN16)

    ctx_sb = ctxp.tile([128, HL, NC_, VL + 1], F32, name="ctx_sb")
    out_sb = outp.tile([128, NC_, HL * VL], BF16, name="out_sb")

    def mm_tile():
        return pp_mm.tile([128, 1024], F32, name="sc")

    def ppart(bias_d, dtype=F32):
        t = const.tile([128, 1], dtype, name=f"b_{bias_d.name}")
        nc.scalar.dma_start(out=t, in_=bias_d.unsqueeze(-1))
        return t

    # ------- prologue: xT half-block stream + k/q projection chunks -------
    # 256-col half-blocks, double-buffered so the PE never starves (and keeps
    # its p-state ramped); weight DMAs interleave between the x transfers
    def emit_qk_chunk(xp, wl, wcl, bl, bcl, dest, hb):
        ps = mm_tile()
        for d in range(DC):
            nc.tensor.matmul(ps[:, 0:256], wl[:, d, :], xp[:, d, :],
                             start=(d == 0), stop=(d == DC - 1))
        for d in range(DC):
            nc.tensor.matmul(ps[:, 512:768], wcl[:, d, :], xp[:, d, :],
                             start=(d == 0), stop=(d == DC - 1))
        sig = sigp.tile([128, 256], F32, name="sg")
        nc.scalar.activation(out=sig, in_=ps[:, 512:768],
                             func=AF.Sigmoid, bias=bcl)
        nc.vector.scalar_tensor_tensor(
            out=dest[:, hb * 256:(hb + 1) * 256], in0=ps[:, 0:256], scalar=bl,
            in1=sig, op0=ALU.add, op1=ALU.mult)

    bq_sb = ppart(bq_d)
    bqc_sb = ppart(bqc_d)
    bk_sb = ppart(bk_d)
    bkc_sb = ppart(bkc_d)
    bv_row = const.tile([1, 512], F32R, name="bv_row")
    nc.scalar.dma_start(out=bv_row, in_=bv_d.unsqueeze(0))
    bvc_row = const.tile([1, 512], F32R, name="bvc_row")
    nc.scalar.dma_start(out=bvc_row, in_=bvc_d.unsqueeze(0))
    nc.scalar.dma_start(out=bdiag, in_=bdiag_d)

    wrv = wv_d.rearrange("(a p) c -> p a c", p=128)
    wrvc = wvc_d.rearrange("(a p) c -> p a c", p=128)
    nc.sync.dma_start(out=wk, in_=wk_d.rearrange("(a p) c -> p a c", p=128))
    nc.scalar.dma_start(out=wkc, in_=wkc_d.rearrange("(a p) c -> p a c", p=128))
    for hb in range(8):
        xp = xTroll.tile([128, DC, 256], F32R, name="xTp", bufs=2)
        for i in range(2):
            eng = nc.sync if i == 0 else nc.scalar
            lo = i * 128
            eng.dma_start(out=xp[:, :, lo:lo + 128],
                          in_=xTr[:, :, hb * 256 + lo:hb * 256 + lo + 128])
        if hb == 0:
            nc.sync.dma_start(out=wq, in_=wq_d.rearrange("(a p) c -> p a c", p=128))
            nc.scalar.dma_start(out=wqc, in_=wqc_d.rearrange("(a p) c -> p a c", p=128))
        elif hb >= 1 and hb <= 4:
            for d in range(2 * (hb - 1), 2 * (hb - 1) + 2):
                nc.sync.dma_start(out=wv[:, d, :], in_=wrv[:, d, :])
                nc.scalar.dma_start(out=wvc[:, d, :], in_=wrvc[:, d, :])
        emit_qk_chunk(xp, wk, wkc, bk_sb, bkc_sb, kT, hb)
        emit_qk_chunk(xp, wq, wqc, bq_sb, bqc_sb, qT, hb)

    # ---- pad constants from biases (pad x rows are zero) ----
    sgk = sigp.tile([128, 1], F32, name="sgk")
    nc.scalar.activation(out=sgk, in_=bkc_sb, func=AF.Sigmoid)
    pkf = sigp.tile([128, 1], F32, name="pkf")
    nc.gpsimd.tensor_tensor(out=pkf, in0=bk_sb, in1=sgk, op=ALU.mult)
    nc.gpsimd.tensor_scalar(out=padk, in0=bdiag, scalar1=pkf,
                            scalar2=None, op0=ALU.mult)
    # pad-v row: bv * sigmoid(bvc), replicated to partitions 32/64
    sgv = sigp.tile([128, 512], F32, name="sg")
    nc.scalar.activation(out=sgv[0:1, :], in_=bvc_row.bitcast(F32), func=AF.Sigmoid)
    nc.gpsimd.tensor_tensor(
        out=vt[0:1, NT, :, 0:VL],
        in0=bv_row.bitcast(F32).rearrange("p (h v) -> p h v", h=HL),
        in1=sgv[0:1, :].rearrange("p (h v) -> p h v", h=HL),
        op=ALU.mult)
    # ones column for the softmax denominator, all tiles incl. pad
    nc.gpsimd.memset(vt[:, :, :, VL:VL + 1], 1.0)
    for bp in (32, 64):
        nc.sync.dma_start(out=vt[bp:bp + 1, NT, :, :], in_=vt[0:1, NT, :, :])

    # ---- e_pad (first exps; ACT switches tables once right before) ----
    for p in range(NP):
        pps = mm_tile()
        for j in range(QW // 512):
            nc.tensor.matmul(pps[0:HL, j * 512:(j + 1) * 512], padk,
                             qT[:, p * QW + j * 512: p * QW + (j + 1) * 512],
                             start=True, stop=True)
        nc.scalar.activation(out=e_pad[0:HL, p, :], in_=pps[0:HL, :],
                             func=AF.Exp, scale=SCALE, bias=ln16_sb[0:HL, :])

    def stage_ep(p):
        ept[p] = epadp.tile([128, 3, QW], BF16, name="ept", bufs=2)
        for h in range(HL):
            nc.scalar.dma_start(
                out=ept[p][32 * (h // 3):32 * (h // 3) + 1, h % 3, :],
                in_=e_pad[h:h + 1, p, :])

    # ---- heads whose 16h base partition is not in {0,32,64} get restaged ----
    # direct: h 0,2,4; tile A: h 1,3,5 at 0/32/64; tile B: h 6,7 at 0/32
    kh_a = stg.tile([128, S], F32R, name="kh_a")
    kh_b = stg.tile([128, S], F32R, name="kh_b")
    qhs = {}

    def _stage_base(h):
        if h in (0, 2, 4):
            return None, 16 * h
        if h in (1, 3, 5):
            return 0, 32 * ((h - 1) // 2)
        return 1, 32 * (h - 6)

    for h in (1, 3, 5, 6, 7):
        ab, bp = _stage_base(h)
        nc.sync.dma_start(out=(kh_a, kh_b)[ab][bp:bp + QL, :],
                          in_=kT[h * QL:(h + 1) * QL, :])

    def stage_qh(p):
        eng = nc.sync if p == 0 else nc.scalar
        qa = stg.tile([128, QW], F32R, name="qh_a", bufs=2)
        qb = stg.tile([128, QW], F32R, name="qh_b", bufs=2)
        qhs[p] = (qa, qb)
        for h in (1, 3, 5, 6, 7):
            ab, bp = _stage_base(h)
            eng.dma_start(out=(qa, qb)[ab][bp:bp + QL, :],
                          in_=qT[h * QL:(h + 1) * QL, p * QW:(p + 1) * QW])


    def kh_ap(h, t):
        ab, bp = _stage_base(h)
        src = kT if ab is None else (kh_a, kh_b)[ab]
        return src[bp:bp + QL, t * 128:(t + 1) * 128]

    def qh_ap(h, p, j):
        ab, bp = _stage_base(h)
        if ab is None:
            return qT[bp:bp + QL, p * QW + j * 512:p * QW + j * 512 + 512]
        return qhs[p][ab][bp:bp + QL, j * 512:(j + 1) * 512]

    # ---------------- attention passes ----------------
    wpool.release()
    xT_small = {}

    def load_xT_tile(t):
        """Re-stream xT tile t for the v projections (2-slot window)."""
        xT_small[t] = xTroll.tile([128, DC, 128], F32R, name="xTv", bufs=2)
        nc.sync.dma_start(out=xT_small[t],
                          in_=xTr[:, :, t * 128:(t + 1) * 128])

    def emit_v_tile(t):
        """v'[t] = [(lin+bv) * 1/(1+exp(-(linc+bvc))) | 1], token-major bf16."""
        xp = xT_small.pop(t)
        ps = pp_v.tile([128, 1024], F32, name="vps")
        for d in range(DC):
            nc.tensor.matmul(ps[:, 0:512], xp[:, d, :], wv[:, d, :],
                             start=(d == 0), stop=False)
        nc.tensor.matmul(ps[:, 0:512], ones_col, bv_row, start=False, stop=True)
        for d in range(DC):
            nc.tensor.matmul(ps[:, 512:1024], xp[:, d, :], wvc[:, d, :],
                             start=(d == 0), stop=False)
        nc.tensor.matmul(ps[:, 512:1024], ones_col, bvc_row, start=False, stop=True)
        eg = sigp.tile([128, 512], F32, name="sg")
        nc.scalar.activation(out=eg, in_=ps[:, 512:1024], func=AF.Exp, scale=-1.0)
        ug = sigp.tile([128, 512], F32, name="sg")
        nc.gpsimd.tensor_scalar(out=ug, in0=eg, scalar1=1.0, scalar2=None,
                                op0=ALU.add)
        rg = sigp.tile([128, 512], F32, name="sg")
        nc.vector.reciprocal(out=rg, in_=ug)
        nc.vector.tensor_tensor(
            out=vt[:, t, :, 0:VL],
            in0=ps[:, 0:512].rearrange("p (h v) -> p h v", h=HL),
            in1=rg.rearrange("p (h v) -> p h v", h=HL),
            op=ALU.mult)

    load_xT_tile(0)
    stage_qh(0)
    stage_ep(0)
    stage_ep(1)
    stage_qh(1)
    etmp.release()
    emit_v_tile(0)
    load_xT_tile(1)

    def emit_ctx(p, t, h, et):
        for half, pool, eng in ((0, pp_ctxa, nc.vector), (1, pp_ctxb, nc.vector)):
            pt = pool.tile([128, NC_ // 2, VL + 1], F32, name="pt")
            for cl in range(NC_ // 2):
                c = half * (NC_ // 2) + cl
                if t == 0:
                    bp = 32 * (h // 3)
                    nc.tensor.matmul(pt[:, cl, :],
                                     ept[p][bp:bp + 1, h % 3, c * 128:(c + 1) * 128],
                                     vt[bp:bp + 1, NT, h, :],
                                     start=True, stop=False)
                    nc.tensor.matmul(pt[:, cl, :],
                                     et[:, c * 128:(c + 1) * 128],
                                     vt[:, t, h, :], start=False, stop=True)
                else:
                    nc.tensor.matmul(pt[:, cl, :],
                                     et[:, c * 128:(c + 1) * 128],
                                     vt[:, t, h, :], start=True, stop=True)
            dst = ctx_sb[:, h, half * (NC_ // 2):(half + 1) * (NC_ // 2), :]
            if t == 0:
                eng.tensor_copy(out=dst, in_=pt)
            else:
                eng.tensor_tensor(out=dst, in0=dst, in1=pt, op=ALU.add)

    def emit_epilogue(p, h):
        rc = rcp.tile([128, HL, 1], F32, name="rc")
        nc.vector.reciprocal(out=rc[:, :, 0], in_=ctx_sb[:, h, :, VL])
        for c in range(NC_):
            nc.gpsimd.tensor_scalar(
                out=out_sb[:, c, h * VL:(h + 1) * VL],
                in0=ctx_sb[:, h, c, 0:VL],
                scalar1=rc[:, c, :], scalar2=None, op0=ALU.mult)

    def flush_out(p, h):
        yr = y_d[p * QW:(p + 1) * QW, :].rearrange("(a p) c -> p a c", p=128)
        if h == 3:
            nc.sync.dma_start(out=yr[:, :, 0:4 * VL],
                              in_=out_sb[:, :, 0:4 * VL])
        elif h == HL - 2:
            nc.sync.dma_start(out=yr[:, :, 4 * VL:(HL - 1) * VL],
                              in_=out_sb[:, :, 4 * VL:(HL - 1) * VL])
        elif h == HL - 1:
            nc.sync.dma_start(out=yr[:, :, (HL - 1) * VL:],
                              in_=out_sb[:, :, (HL - 1) * VL:])

    for p in range(NP):
        pend = []
        for t in range(NT):
            for h in (0, 2, 4, 1, 3, 5, 6, 7):
                sc = pp_mm.tile([128, QW], F32, name="sc")
                for j in range(QW // 512):
                    nc.tensor.matmul(sc[:, j * 512:(j + 1) * 512],
                                     kh_ap(h, t), qh_ap(h, p, j),
                                     start=True, stop=True)
                et = expp.tile([128, QW], BF16, name="et")
                nc.scalar.activation(out=et, in_=sc, func=AF.Exp, scale=SCALE)
                pend.append((t, h, et))
                if len(pend) > CTX_LAG:
                    tp_, hp_, ep_ = pend.pop(0)
                    emit_ctx(p, tp_, hp_, ep_)
                    if tp_ == NT - 1:
                        emit_epilogue(p, hp_)
                        flush_out(p, hp_)
                # v tile t+1 pipelines one slot ahead of its first consumer
                if p == 0 and t < NT - 1:
                    if h == 1:
                        emit_v_tile(t + 1)
                    elif h == 4 and t < NT - 2:
                        load_xT_tile(t + 2)
        for tp_, hp_, ep_ in pend:
            emit_ctx(p, tp_, hp_, ep_)
            emit_epilogue(p, hp_)
            flush_out(p, hp_)
        if p == 0:
            xTroll.release()
            wvpool.release()

    for pool in (pp_v, pp_ctxb, pp_ctxa, pp_mm, outp, rcp, ctxp, expp, stg,
                 sigp, epadp, vtp, proj, const):
        pool.release()


_NC = None


def _get_nc():
    global _NC
    if _NC is None:
        _NC = _build()
    return _NC


def _shard_inputs(inputs):
    x = np.ascontiguousarray(np.asarray(inputs["x"], dtype=np.float32))
    bdiag = np.repeat(np.eye(8, dtype=np.float32), 16, axis=0)
    in_maps = []
    for c in range(8):
        b, hg = c // 2, c % 2
        qk = slice(hg * 128, (hg + 1) * 128)
        vv = slice(hg * 512, (hg + 1) * 512)
        in_maps.append({
            "xT": np.ascontiguousarray(x[b].T),
            "bdiag": bdiag,
            "wq": np.ascontiguousarray(inputs["Wq"][:, qk]),
            "wqc": np.ascontiguousarray(inputs["Wqc"][:, qk]),
            "wk": np.ascontiguousarray(inputs["Wk"][:, qk]),
            "wkc": np.ascontiguousarray(inputs["Wkc"][:, qk]),
            "wv": np.ascontiguousarray(inputs["Wv"][:, vv]),
            "wvc": np.ascontiguousarray(inputs["Wvc"][:, vv]),
            "bq": np.ascontiguousarray(inputs["bq"][qk]),
            "bqc": np.ascontiguousarray(inputs["bqc"][qk]),
            "bk": np.ascontiguousarray(inputs["bk"][qk]),
            "bkc": np.ascontiguousarray(inputs["bkc"][qk]),
            "bv": np.ascontiguousarray(inputs["bv"][vv]),
            "bvc": np.ascontiguousarray(inputs["bvc"][vv]),
        })
    return in_maps


def kernel(**inputs) -> np.ndarray:
    nc = _get_nc()
    in_maps = _shard_inputs(inputs)
    res = run_bass_kernel_spmd(nc, in_maps, list(range(8)))
    out = np.empty((B, S, 1024), np.float32)
    for c in range(8):
        b, hg = c // 2, c % 2
        out[b, :, hg * 512:(hg + 1) * 512] = np.asarray(
            res.results[c]["y"], dtype=np.float32)
    return out


if __name__ == "__main__":
    rng = np.random.default_rng(0)
    d = 1.0 / np.sqrt(D)
    inputs = {
        "x": rng.standard_normal((B, S, D), dtype=np.float32),
        "Wq": rng.standard_normal((D, 256), dtype=np.float32) * d,
        "bq": rng.standard_normal(256).astype(np.float32) * 0.02,
        "Wqc": rng.standard_normal((D, 256), dtype=np.float32) * d,
        "bqc": rng.standard_normal(256).astype(np.float32) * 0.02,
        "Wk": rng.standard_normal((D, 256), dtype=np.float32) * d,
        "bk": rng.standard_normal(256).astype(np.float32) * 0.02,
        "Wkc": rng.standard_normal((D, 256), dtype=np.float32) * d,
        "bkc": rng.standard_normal(256).astype(np.float32) * 0.02,
        "Wv": rng.standard_normal((D, 1024), dtype=np.float32) * d,
        "bv": rng.standard_normal(1024).astype(np.float32) * 0.02,
        "Wvc": rng.standard_normal((D, 1024), dtype=np.float32) * d,
        "bvc": rng.standard_normal(1024).astype(np.float32) * 0.02,
    }
    y = kernel(**inputs)
    print("kernel output", y.shape, y.dtype, float(np.abs(y).max()))
